# revision 13
# baseline (speedup 1.0000x reference)
"""Trainium2 Bass kernel for CTC loss (K.ctc_batch_cost semantics).

Problem (hardcoded): B=1024, T=256, C=128, L=32, blank=C-1, S=2L+1=65.
Sharding: pure data parallel, 128 examples per core across 8 cores.

Device algorithm (per core) — "state sweep" over a constant-boosted
linear-domain DP:

    alpha_hat[t,s] = (y[t-1,s] + alpha_hat[t-1,s]) * E[t,s]
    y[t,s]   = alpha_hat[t,s-1] + m2[s]*alpha_hat[t,s-2]   (odd s>=3)
    y[t,s]   = alpha_hat[t,s-1]                            (even s, s=1)
    E[t,s]   = (p[b,t,ext[s]] + eps) * K_w

K_w is a per-window constant boost countering the natural ~e^-4.5/step
decay of alpha (drift measured empirically per window; residual
random-walk stays within fp32/bf16 range).  One data-dependent
re-anchor (divide by per-example max) at the window boundary.

    loss[b] = -ln(fin) - ln(mx) + TW*(lnK0+lnK1)

Per 128-step window: stream y_pred t-major ([t-partitions, (e,c)]),
gpsimd ap_gather of the 65 needed columns per example (s-major), PE
transpose per state (strided gather-output view, 4 states per PSUM
tile), ACT drain fused with the *K_w+eps*K_w affine into bf16 E.
Sweep: 65 DVE scans + 31 odd-state scalar_tensor_tensor per window,
chained in-order on DVE; scans start as soon as their 4-state E chunk
is drained.
"""

import numpy as np

EPS = 1e-7
LNK = (4.19, 4.73)            # per-window ln-boost (empirically tuned)
B_TOT, T, C, L = 1024, 256, 128, 32
NCORES = 8
B = B_TOT // NCORES          # 128 examples per core
S = 2 * L + 1                # 65
TW = 128                     # window size (time steps)
NWIN = T // TW               # 2
EG = 32                      # examples per gather op
NGRP = B // EG               # 4 gather groups
NIDX = EG * S                # 2080 gather indices per op
SER = T + 1                  # series cols per state (col 0 == t=-1)

K_F32 = tuple(np.float32(np.exp(k)) for k in LNK)
LNK_TOT = float(sum(TW * np.log(np.float64(k)) for k in K_F32))

_CACHE = {}


# ----------------------------------------------------------------------------
# host-side tables
# ----------------------------------------------------------------------------

def _host_tables(y_true):
    """Wrapped s-major gather tables per core.

    Layout per core: 4 full-group tables (NIDX/16 cols each).
    """
    lab = np.asarray(y_true).astype(np.int32)
    ext = np.full((B_TOT, S), C - 1, np.int32)
    ext[:, 1::2] = lab

    def wrap(b0, ne):
        # s-major: flat[s*ne + e] = e*C + ext[b0+e, s]
        flat = (np.arange(ne)[None, :] * C + ext[b0:b0 + ne].T).reshape(-1)
        wrapped = flat.reshape(-1, 16).T                  # [16, ne*S/16]
        return np.tile(wrapped, (8, 1)).astype(np.int16)  # [128, ne*S/16]

    out = np.zeros((NCORES, 128, NGRP * (NIDX // 16)), np.int16)
    for core in range(NCORES):
        cols = [wrap(core * B + g * EG, EG) for g in range(NGRP)]
        out[core] = np.concatenate(cols, axis=1)
    return np.ascontiguousarray(out)


def _host_mask2(y_true):
    """m2[b, s] = 1 if skip into odd state s>=3 allowed, else 0. [B_TOT, S]."""
    lab = np.asarray(y_true).astype(np.int32)
    m2 = np.zeros((B_TOT, S), np.float32)
    m2[:, 3::2] = (lab[:, 1:] != lab[:, :-1]).astype(np.float32)
    return m2


# ----------------------------------------------------------------------------
# device kernel
# ----------------------------------------------------------------------------

def _build_module():
    import concourse.bacc as bacc
    import concourse.mybir as mybir
    import concourse.tile as tile
    from concourse import library_config
    from concourse.tile_rust import add_dep_helper

    dt = mybir.dt
    AX = mybir.AxisListType
    AF = mybir.ActivationFunctionType
    OP = mybir.AluOpType

    nc = bacc.Bacc("TRN2", target_bir_lowering=False, debug=False,
                   enable_asserts=False, num_devices=NCORES)

    yp = nc.dram_tensor("y_pred", [B, T, C], dt.float32, kind="ExternalInput")
    gtab = nc.dram_tensor("gtab", [128, NGRP * (NIDX // 16)], dt.int16,
                          kind="ExternalInput")
    m2_in = nc.dram_tensor("m2", [B, S], dt.float32, kind="ExternalInput")
    ident_in = nc.dram_tensor("ident", [128, 128], dt.bfloat16,
                              kind="ExternalInput")
    fin_out = nc.dram_tensor("fin", [B, 1], dt.float32, kind="ExternalOutput")
    mx_out = nc.dram_tensor("mx", [B, 1], dt.float32, kind="ExternalOutput")

    with tile.TileContext(nc) as tc:
        with (
            tc.tile_pool(name="const", bufs=1) as cpool,
            tc.tile_pool(name="pin", bufs=2) as ppool,
            tc.tile_pool(name="eg", bufs=2) as gpool,
            tc.tile_pool(name="ecb", bufs=2) as epool,
            tc.tile_pool(name="ybuf", bufs=3) as ypool,
            tc.tile_pool(name="small", bufs=1) as spool,
            tc.tile_pool(name="tp", bufs=2, space="PSUM") as tpool,
        ):
            ident_sb = cpool.tile([128, 128], dt.bfloat16, name="ident_sb")
            nc.sync.dma_start(ident_sb, ident_in[:, :])
            gtab_sb = cpool.tile([128, NGRP * (NIDX // 16)], dt.int16,
                                 name="gtab_sb")
            nc.sync.dma_start(gtab_sb, gtab[:, :])
            m2_sb = cpool.tile([B, S], dt.float32, name="m2_sb")
            nc.sync.dma_start(m2_sb, m2_in[:, :])

            lib_inst = nc.gpsimd.load_library(library_config.ap_gather)

            # alpha_hat series: [128, S, SER] bf16; col 0 = t=-1 (zeros)
            series = spool.tile([B, S * SER], dt.bfloat16, name="series")
            ser_v = series.rearrange("p (s t) -> p s t", t=SER)
            nc.vector.memset(ser_v[:, :, 0], 0.0)

            zeros_b = spool.tile([B, TW], dt.bfloat16, name="zeros_b")
            nc.vector.memset(zeros_b, 0.0)

            mx = spool.tile([B, 1], dt.float32, name="mx")
            rr = spool.tile([B, 1], dt.float32, name="rr")
            fin = spool.tile([B, 1], dt.float32, name="fin")

            ecomb = []
            for w in range(NWIN):
                e_t = epool.tile([B, S * TW], dt.bfloat16, tag="ecomb",
                                 name=f"ecomb{w}")
                ecomb.append(e_t)

            def prep_window(w):
                """DMA + gather + transpose + K-boost drain for window w."""
                t0 = w * TW
                kw = float(K_F32[w])
                egath = gpool.tile([128, NGRP * NIDX], dt.float32, tag="eg",
                                   name=f"egath{w}")
                egath2 = gpool.tile([128, S * B], dt.bfloat16, tag="eg2",
                                    name=f"egath2_{w}")
                eg2v = egath2.rearrange("p (s b) -> p s b", s=S)
                # (example-offset, n-examples, idx-table col, egath col)
                segs = [(g * EG, EG, g * (NIDX // 16), g * NIDX)
                        for g in range(NGRP)]
                for si, (b0, ne, ic, ec) in enumerate(segs):
                    nidx = ne * S
                    ptile = ppool.tile([128, EG * C], dt.float32,
                                       tag="pt", name=f"pt{w}_{si}")
                    pv = ptile[:, 0:ne * C].rearrange("p (e c) -> p e c", c=C)
                    nc.sync.dma_start(
                        pv,
                        yp[b0:b0 + ne, t0:t0 + TW, :]
                        .rearrange("e t c -> t e c"))
                    gi = nc.gpsimd.ap_gather(
                        egath[:, ec:ec + nidx], ptile[:, 0:ne * C],
                        gtab_sb[:, ic:ic + nidx // 16],
                        channels=128, num_elems=ne * C, d=1, num_idxs=nidx)
                    add_dep_helper(lib_inst.ins, gi.ins, sync=False,
                                   reason="library before gather")
                    # per-group rearrange to s-major, fused with the
                    # *K + eps*K affine and bf16 convert (pre-transpose,
                    # so recips/divides never appear downstream).
                    nc.scalar.activation(
                        eg2v[:, :, b0:b0 + ne],
                        egath[:, ec:ec + nidx]
                        .rearrange("p (s e) -> p s e", s=S),
                        AF.Copy, bias=float(EPS) * kw, scale=kw)
                # per-state PE transpose (bf16, 4 states per PSUM tile),
                # pure-copy ACT drain into bf16 E.
                ecv = ecomb[w]
                for s0 in range(0, S, 4):
                    ns = min(4, S - s0)
                    tp = tpool.tile([128, 4 * TW], dt.bfloat16, tag="tp",
                                    name=f"tp{w}_{s0}")
                    for k in range(ns):
                        nc.tensor.transpose(
                            tp[:, k * TW:(k + 1) * TW],
                            eg2v[:, s0 + k, :], ident_sb)
                    nc.scalar.activation(
                        ecv[:, s0 * TW:(s0 + ns) * TW],
                        tp[:, 0:ns * TW], AF.Copy)

            def sweep_window(w):
                """Run the s-sweep scans for window w."""
                t0 = w * TW
                ecv = ecomb[w].rearrange("p (s t) -> p s t", t=TW)
                for s in range(S):
                    out_ap = ser_v[:, s, t0 + 1:t0 + 1 + TW]
                    if w == 0:
                        init = 1.0 if s <= 1 else 0.0
                    else:
                        init = ser_v[:, s, t0:t0 + 1]
                    if s == 0:
                        d0 = zeros_b
                    elif s % 2 == 0 or s == 1:
                        d0 = ser_v[:, s - 1, t0:t0 + TW]
                    else:
                        yb = ypool.tile([B, TW], dt.bfloat16, tag="yb",
                                        name=f"yb{w}_{s}")
                        nc.vector.scalar_tensor_tensor(
                            yb, ser_v[:, s - 2, t0:t0 + TW],
                            m2_sb[:, s:s + 1], ser_v[:, s - 1, t0:t0 + TW],
                            op0=OP.mult, op1=OP.add)
                        d0 = yb
                    nc.vector.tensor_tensor_scan(
                        out_ap, d0, ecv[:, s, :], init,
                        op0=OP.add, op1=OP.mult)

                if w < NWIN - 1:
                    # re-anchor: divide all states' boundary col by max
                    bv = ser_v[:, :, t0 + TW]
                    nc.vector.tensor_reduce(mx, bv, axis=AX.X, op=OP.max)
                    nc.vector.reciprocal(rr, mx)
                    nc.vector.tensor_scalar_mul(bv, bv, rr)
                    nc.sync.dma_start(mx_out[:, :], mx)

            # schedule: prep w0, prep w1 (engines pipeline via deps),
            # sweep w0, sweep w1
            for w in range(NWIN):
                prep_window(w)
            for w in range(NWIN):
                sweep_window(w)

            nc.vector.tensor_add(fin, ser_v[:, S - 2, T:T + 1],
                                 ser_v[:, S - 1, T:T + 1])
            nc.sync.dma_start(fin_out[:, :], fin)

    nc.compile()
    return nc


def _get_module():
    if "nc" not in _CACHE:
        _CACHE["nc"] = _build_module()
    return _CACHE["nc"]


# ----------------------------------------------------------------------------
# entry point
# ----------------------------------------------------------------------------

def _feeds(y_true, y_pred):
    import ml_dtypes
    y_pred = np.ascontiguousarray(np.asarray(y_pred, dtype=np.float32))
    tables = _host_tables(y_true)
    m2 = _host_mask2(y_true)
    ident = np.eye(128, dtype=ml_dtypes.bfloat16)
    maps = []
    for core in range(NCORES):
        maps.append({
            "y_pred": y_pred[core * B:(core + 1) * B],
            "gtab": tables[core],
            "m2": m2[core * B:(core + 1) * B],
            "ident": ident,
        })
    return maps


def _run(y_true, y_pred, trace=False):
    from concourse.bass_utils import run_bass_kernel_spmd
    nc = _get_module()
    return run_bass_kernel_spmd(nc, _feeds(y_true, y_pred),
                                core_ids=list(range(NCORES)), trace=trace)


def kernel(y_true, y_pred):
    res = _run(y_true, y_pred)
    out = np.zeros(B_TOT, np.float64)
    for i in range(NCORES):
        fin = res.results[i]["fin"].reshape(B).astype(np.float64)
        mxv = res.results[i]["mx"].reshape(B).astype(np.float64)
        out[i * B:(i + 1) * B] = LNK_TOT - np.log(mxv) - np.log(fin)
    return out.astype(np.float32)[:, None]


def profile_once(y_true, y_pred):
    res = _run(y_true, y_pred, trace=True)
    return res.exec_time_ns


if __name__ == "__main__":
    rng = np.random.default_rng(0)
    yt = rng.integers(0, 126, size=(B_TOT, L)).astype(np.int64)
    logits = rng.standard_normal((B_TOT, T, C)).astype(np.float32)
    ex = np.exp(logits - logits.max(-1, keepdims=True))
    ypred = (ex / ex.sum(-1, keepdims=True)).astype(np.float32)
    out = kernel(yt, ypred)
    print("out", out.shape, out[:4, 0])


# revision 14
# speedup vs baseline: 1.0065x; 1.0065x over previous
"""Trainium2 Bass kernel for CTC loss (K.ctc_batch_cost semantics).

Problem (hardcoded): B=1024, T=256, C=128, L=32, blank=C-1, S=2L+1=65.
Sharding: pure data parallel, 128 examples per core across 8 cores.

Device algorithm (per core) — "state sweep" over a constant-boosted
linear-domain DP:

    alpha_hat[t,s] = (y[t-1,s] + alpha_hat[t-1,s]) * E[t,s]
    y[t,s]   = alpha_hat[t,s-1] + m2[s]*alpha_hat[t,s-2]   (odd s>=3)
    y[t,s]   = alpha_hat[t,s-1]                            (even s, s=1)
    E[t,s]   = (p[b,t,ext[s]] + eps) * K_w

K_w is a per-window constant boost countering the natural ~e^-4.5/step
decay of alpha (drift measured empirically per window; residual
random-walk stays within fp32/bf16 range).  One data-dependent
re-anchor (divide by per-example max) at the window boundary.

    loss[b] = -ln(fin) - ln(mx) + TW*(lnK0+lnK1)

Per 128-step window: stream y_pred t-major ([t-partitions, (e,c)]),
gpsimd ap_gather of the 65 needed columns per example (s-major), PE
transpose per state (strided gather-output view, 4 states per PSUM
tile), ACT drain fused with the *K_w+eps*K_w affine into bf16 E.
Sweep: 65 DVE scans + 31 odd-state scalar_tensor_tensor per window,
chained in-order on DVE; scans start as soon as their 4-state E chunk
is drained.
"""

import numpy as np

EPS = 1e-7
LNK = (4.19, 4.73)            # per-window ln-boost (empirically tuned)
B_TOT, T, C, L = 1024, 256, 128, 32
NCORES = 8
B = B_TOT // NCORES          # 128 examples per core
S = 2 * L + 1                # 65
TW = 128                     # window size (time steps)
NWIN = T // TW               # 2
EG = 32                      # examples per gather op
NGRP = B // EG               # 4 gather groups
NIDX = EG * S                # 2080 gather indices per op
SER = T + 1                  # series cols per state (col 0 == t=-1)

K_F32 = tuple(np.float32(np.exp(k)) for k in LNK)
LNK_TOT = float(sum(TW * np.log(np.float64(k)) for k in K_F32))

_CACHE = {}


# ----------------------------------------------------------------------------
# host-side tables
# ----------------------------------------------------------------------------

def _host_tables(y_true):
    """Wrapped s-major gather tables per core.

    Layout per core: 4 full-group tables (NIDX/16 cols each).
    """
    lab = np.asarray(y_true).astype(np.int32)
    ext = np.full((B_TOT, S), C - 1, np.int32)
    ext[:, 1::2] = lab

    def wrap(b0, ne):
        # s-major: flat[s*ne + e] = e*C + ext[b0+e, s]
        flat = (np.arange(ne)[None, :] * C + ext[b0:b0 + ne].T).reshape(-1)
        wrapped = flat.reshape(-1, 16).T                  # [16, ne*S/16]
        return np.tile(wrapped, (8, 1)).astype(np.int16)  # [128, ne*S/16]

    out = np.zeros((NCORES, 128, NGRP * (NIDX // 16)), np.int16)
    for core in range(NCORES):
        cols = [wrap(core * B + g * EG, EG) for g in range(NGRP)]
        out[core] = np.concatenate(cols, axis=1)
    return np.ascontiguousarray(out)


def _host_mask2(y_true):
    """m2[b, s] = 1 if skip into odd state s>=3 allowed, else 0. [B_TOT, S]."""
    lab = np.asarray(y_true).astype(np.int32)
    m2 = np.zeros((B_TOT, S), np.float32)
    m2[:, 3::2] = (lab[:, 1:] != lab[:, :-1]).astype(np.float32)
    return m2


# ----------------------------------------------------------------------------
# device kernel
# ----------------------------------------------------------------------------

def _build_module():
    import concourse.bacc as bacc
    import concourse.mybir as mybir
    import concourse.tile as tile
    from concourse import library_config
    from concourse.tile_rust import add_dep_helper

    dt = mybir.dt
    AX = mybir.AxisListType
    AF = mybir.ActivationFunctionType
    OP = mybir.AluOpType

    nc = bacc.Bacc("TRN2", target_bir_lowering=False, debug=False,
                   enable_asserts=False, num_devices=NCORES)

    yp = nc.dram_tensor("y_pred", [B, T, C], dt.float32, kind="ExternalInput")
    gtab = nc.dram_tensor("gtab", [128, NGRP * (NIDX // 16)], dt.int16,
                          kind="ExternalInput")
    m2_in = nc.dram_tensor("m2", [B, S], dt.float32, kind="ExternalInput")
    ident_in = nc.dram_tensor("ident", [128, 128], dt.bfloat16,
                              kind="ExternalInput")
    fin_out = nc.dram_tensor("fin", [B, 1], dt.float32, kind="ExternalOutput")
    mx_out = nc.dram_tensor("mx", [B, 1], dt.float32, kind="ExternalOutput")

    with tile.TileContext(nc) as tc:
        with (
            tc.tile_pool(name="const", bufs=1) as cpool,
            tc.tile_pool(name="pin", bufs=2) as ppool,
            tc.tile_pool(name="eg", bufs=2) as gpool,
            tc.tile_pool(name="ecb", bufs=2) as epool,
            tc.tile_pool(name="ybuf", bufs=3) as ypool,
            tc.tile_pool(name="small", bufs=1) as spool,
            tc.tile_pool(name="tp", bufs=2, space="PSUM") as tpool,
        ):
            ident_sb = cpool.tile([128, 128], dt.bfloat16, name="ident_sb")
            nc.sync.dma_start(ident_sb, ident_in[:, :])
            gtab_sb = cpool.tile([128, NGRP * (NIDX // 16)], dt.int16,
                                 name="gtab_sb")
            nc.sync.dma_start(gtab_sb, gtab[:, :])
            m2_sb = cpool.tile([B, S], dt.float32, name="m2_sb")
            nc.sync.dma_start(m2_sb, m2_in[:, :])

            lib_inst = nc.gpsimd.load_library(library_config.ap_gather)

            # alpha_hat series: [128, S, SER] bf16; col 0 = t=-1 (zeros)
            series = spool.tile([B, S * SER], dt.bfloat16, name="series")
            ser_v = series.rearrange("p (s t) -> p s t", t=SER)
            nc.vector.memset(ser_v[:, :, 0], 0.0)

            zeros_b = spool.tile([B, TW], dt.bfloat16, name="zeros_b")
            nc.vector.memset(zeros_b, 0.0)

            mx = spool.tile([B, 1], dt.float32, name="mx")
            rr = spool.tile([B, 1], dt.float32, name="rr")
            fin = spool.tile([B, 1], dt.float32, name="fin")

            ecomb = []
            for w in range(NWIN):
                e_t = epool.tile([B, S * TW], dt.bfloat16, tag="ecomb",
                                 name=f"ecomb{w}")
                ecomb.append(e_t)

            def prep_window(w):
                """DMA + gather + transpose + K-boost drain for window w."""
                t0 = w * TW
                kw = float(K_F32[w])
                egath = gpool.tile([128, NGRP * NIDX], dt.float32, tag="eg",
                                   name=f"egath{w}")
                egath2 = gpool.tile([128, S * B], dt.bfloat16, tag="eg2",
                                    name=f"egath2_{w}")
                eg2v = egath2.rearrange("p (s b) -> p s b", s=S)
                # (example-offset, n-examples, idx-table col, egath col)
                segs = [(g * EG, EG, g * (NIDX // 16), g * NIDX)
                        for g in range(NGRP)]
                def rearr(b0, ne, ec, s_lo, s_hi):
                    # rearrange to s-major, fused with the *K + eps*K
                    # affine and bf16 convert (pre-transpose, so recips/
                    # divides never appear downstream).
                    nc.scalar.activation(
                        eg2v[:, s_lo:s_hi, b0:b0 + ne],
                        egath[:, ec + s_lo * ne:ec + s_hi * ne]
                        .rearrange("p (s e) -> p s e", s=s_hi - s_lo),
                        AF.Copy, bias=float(EPS) * kw, scale=kw)

                ecv = ecomb[w]

                def chunk(s0):
                    # per-state PE transpose (bf16, 4 states per PSUM
                    # tile), pure-copy ACT drain into bf16 E.
                    ns = min(4, S - s0)
                    tp = tpool.tile([128, 4 * TW], dt.bfloat16, tag="tp",
                                    name=f"tp{w}_{s0}")
                    for k in range(ns):
                        nc.tensor.transpose(
                            tp[:, k * TW:(k + 1) * TW],
                            eg2v[:, s0 + k, :], ident_sb)
                    nc.scalar.activation(
                        ecv[:, s0 * TW:(s0 + ns) * TW],
                        tp[:, 0:ns * TW], AF.Copy)

                for si, (b0, ne, ic, ec) in enumerate(segs):
                    nidx = ne * S
                    ptile = ppool.tile([128, EG * C], dt.float32,
                                       tag="pt", name=f"pt{w}_{si}")
                    pv = ptile[:, 0:ne * C].rearrange("p (e c) -> p e c", c=C)
                    nc.sync.dma_start(
                        pv,
                        yp[b0:b0 + ne, t0:t0 + TW, :]
                        .rearrange("e t c -> t e c"))
                    gi = nc.gpsimd.ap_gather(
                        egath[:, ec:ec + nidx], ptile[:, 0:ne * C],
                        gtab_sb[:, ic:ic + nidx // 16],
                        channels=128, num_elems=ne * C, d=1, num_idxs=nidx)
                    add_dep_helper(lib_inst.ins, gi.ins, sync=False,
                                   reason="library before gather")
                    if not (w == NWIN - 1 and si == len(segs) - 1):
                        rearr(b0, ne, ec, 0, S)

                if w == NWIN - 1:
                    # last window: rearrange the final group's first 4
                    # states and emit drain chunk 0 ahead of the rest, so
                    # the sweep chain starts ~2us earlier (ACT queue is
                    # in-order; the full rearrange would sit in front of
                    # the first drain).
                    b0, ne, ic, ec = segs[-1]
                    rearr(b0, ne, ec, 0, 4)
                    chunk(0)
                    rearr(b0, ne, ec, 4, S)
                    for s0 in range(4, S, 4):
                        chunk(s0)
                else:
                    for s0 in range(0, S, 4):
                        chunk(s0)

            def sweep_window(w):
                """Run the s-sweep scans for window w."""
                t0 = w * TW
                ecv = ecomb[w].rearrange("p (s t) -> p s t", t=TW)
                for s in range(S):
                    out_ap = ser_v[:, s, t0 + 1:t0 + 1 + TW]
                    if w == 0:
                        init = 1.0 if s <= 1 else 0.0
                    else:
                        init = ser_v[:, s, t0:t0 + 1]
                    if s == 0:
                        d0 = zeros_b
                    elif s % 2 == 0 or s == 1:
                        d0 = ser_v[:, s - 1, t0:t0 + TW]
                    else:
                        yb = ypool.tile([B, TW], dt.bfloat16, tag="yb",
                                        name=f"yb{w}_{s}")
                        nc.vector.scalar_tensor_tensor(
                            yb, ser_v[:, s - 2, t0:t0 + TW],
                            m2_sb[:, s:s + 1], ser_v[:, s - 1, t0:t0 + TW],
                            op0=OP.mult, op1=OP.add)
                        d0 = yb
                    nc.vector.tensor_tensor_scan(
                        out_ap, d0, ecv[:, s, :], init,
                        op0=OP.add, op1=OP.mult)

                if w < NWIN - 1:
                    # re-anchor: divide all states' boundary col by max
                    bv = ser_v[:, :, t0 + TW]
                    nc.vector.tensor_reduce(mx, bv, axis=AX.X, op=OP.max)
                    nc.vector.reciprocal(rr, mx)
                    nc.vector.tensor_scalar_mul(bv, bv, rr)
                    nc.sync.dma_start(mx_out[:, :], mx)

            # schedule: prep w0, prep w1 (engines pipeline via deps),
            # sweep w0, sweep w1
            for w in range(NWIN):
                prep_window(w)
            for w in range(NWIN):
                sweep_window(w)

            nc.vector.tensor_add(fin, ser_v[:, S - 2, T:T + 1],
                                 ser_v[:, S - 1, T:T + 1])
            nc.sync.dma_start(fin_out[:, :], fin)

    nc.compile()
    return nc


def _get_module():
    if "nc" not in _CACHE:
        _CACHE["nc"] = _build_module()
    return _CACHE["nc"]


# ----------------------------------------------------------------------------
# entry point
# ----------------------------------------------------------------------------

def _feeds(y_true, y_pred):
    import ml_dtypes
    y_pred = np.ascontiguousarray(np.asarray(y_pred, dtype=np.float32))
    tables = _host_tables(y_true)
    m2 = _host_mask2(y_true)
    ident = np.eye(128, dtype=ml_dtypes.bfloat16)
    maps = []
    for core in range(NCORES):
        maps.append({
            "y_pred": y_pred[core * B:(core + 1) * B],
            "gtab": tables[core],
            "m2": m2[core * B:(core + 1) * B],
            "ident": ident,
        })
    return maps


def _run(y_true, y_pred, trace=False):
    from concourse.bass_utils import run_bass_kernel_spmd
    nc = _get_module()
    return run_bass_kernel_spmd(nc, _feeds(y_true, y_pred),
                                core_ids=list(range(NCORES)), trace=trace)


def kernel(y_true, y_pred):
    res = _run(y_true, y_pred)
    out = np.zeros(B_TOT, np.float64)
    for i in range(NCORES):
        fin = res.results[i]["fin"].reshape(B).astype(np.float64)
        mxv = res.results[i]["mx"].reshape(B).astype(np.float64)
        out[i * B:(i + 1) * B] = LNK_TOT - np.log(mxv) - np.log(fin)
    return out.astype(np.float32)[:, None]


def profile_once(y_true, y_pred):
    res = _run(y_true, y_pred, trace=True)
    return res.exec_time_ns


if __name__ == "__main__":
    rng = np.random.default_rng(0)
    yt = rng.integers(0, 126, size=(B_TOT, L)).astype(np.int64)
    logits = rng.standard_normal((B_TOT, T, C)).astype(np.float32)
    ex = np.exp(logits - logits.max(-1, keepdims=True))
    ypred = (ex / ex.sum(-1, keepdims=True)).astype(np.float32)
    out = kernel(yt, ypred)
    print("out", out.shape, out[:4, 0])
